# revision 11
# baseline (speedup 1.0000x reference)
"""Trainium2 Bass kernel for nn_AutoregressiveDecoder (LSTM decoder w/ greedy sampling).

Strategy (8 NeuronCores, SPMD):
  - Vocab-shard the fc projection: core j holds Wfc rows [4000j, 4000(j+1)) padded to
    4096 (pad bias = -1e30), resident in SBUF as bf16 hi/lo pairs (bf16x3 matmuls).
  - x-path precomputation: emb_gates[v] = embedding[v] @ Wih.T + bih + bhh is
    precomputed on the host ([32000, 4, 512] fp32, gate-permuted). Each step the
    x contribution to the gates is a single indirect-DMA gather instead of a
    transpose + 48-matmul pass. Step 0 uses xg0 = h0 @ Wih.T + b (host).
  - LSTM h-weights replicated per core, SBUF-resident, gate columns permuted so that
    PE column-group q computes [i|f|o|g] for hidden quarter q -> full-partition
    elementwise state updates.
  - Greedy token each step: per-half bias-add + max8/max_index pipelined behind the
    vocab matmul, PE-transpose fold over the 4 partition groups, 256B AllGather of
    (val, idx) across cores, arithmetic first-occurrence argmax fold, then
    indirect-DMA gather of the emb_gates row for the next step.
  - Logits stream to DRAM as [T, 128, 1024] per core; host reassembles [B, T, V].
"""
import sys

sys.path.insert(0, "/opt/trn_rl_repo")

import numpy as np

import concourse.bass as bass
import concourse.bacc as bacc
import concourse.tile as tile
import concourse.mybir as mybir
from concourse.bass_utils import run_bass_kernel_spmd

FP32 = mybir.dt.float32
BF16 = mybir.dt.bfloat16
I32 = mybir.dt.int32
U32 = mybir.dt.uint32

N_CORES = 8
B, L, H, E, V, T = 32, 256, 512, 512, 32000, 64
VS = V // N_CORES          # 4000 true shard
VSP = 4096                 # padded shard
BIG = 65536.0

Sigmoid = mybir.ActivationFunctionType.Sigmoid
Tanh = mybir.ActivationFunctionType.Tanh
Alu = mybir.AluOpType


def build_decoder(nc, io, n_steps):
    """Emit the full unrolled decoder. io: dict name -> DRAM AP.
    Returns deferred (instruction, sem, value) waits to attach post-scheduling
    (kept invisible to the Tile scheduler's single-core simulation)."""
    remote_sem = nc.alloc_semaphore("rx_sem")
    local_sem = nc.alloc_semaphore("rdma_ls")
    trig_sem = nc.alloc_semaphore("trig_done")
    deferred = []
    with tile.TileContext(nc) as tc:
        sb = tc.alloc_tile_pool(name="sb", bufs=1)
        sb2 = tc.alloc_tile_pool(name="sb2", bufs=3)
        ps_g = tc.alloc_tile_pool(name="ps_g", bufs=2, space="PSUM")
        ps_v = tc.alloc_tile_pool(name="ps_v", bufs=2, space="PSUM")
        ps_s = tc.alloc_tile_pool(name="ps_s", bufs=2, space="PSUM")
        dr = tc.alloc_tile_pool(name="dr", bufs=4, space="DRAM")
        pools = [sb, sb2, ps_g, ps_v, ps_s, dr]

        # ---- persistent SBUF state & weights ----
        # big matmuls run as bf16x3 (hi*hi + hi*lo + lo*hi, fp32 psum accumulate):
        # ~fp32 precision at 1 cycle/row instead of fp32's 4 cycles/row.
        wfh = [sb.tile([128, VSP], BF16, name=f"wfh{q}") for q in range(4)]
        wfl = [sb.tile([128, VSP], BF16, name=f"wfl{q}") for q in range(4)]
        wgh = [sb.tile([128, 2048], BF16, name=f"wgh{r}") for r in range(4)]
        wgl = [sb.tile([128, 2048], BF16, name=f"wgl{r}") for r in range(4)]
        bias_fcv = sb.tile([128, 1024], FP32, name="bias_fcv")
        goffs = sb.tile([128, 2], FP32, name="goffs")
        ident = sb.tile([128, 128], FP32, name="ident")
        hT = sb.tile([128, 128], FP32, name="hT")
        hTh = sb.tile([128, 128], BF16, name="hTh")
        hTl = sb.tile([128, 128], BF16, name="hTl")
        c_t = sb.tile([128, 128], FP32, name="c_t")

        for q in range(4):
            nc.sync.dma_start(wfh[q][:], io["wfc_hi"][128 * q:128 * (q + 1), :])
            nc.sync.dma_start(wfl[q][:], io["wfc_lo"][128 * q:128 * (q + 1), :])
        for r in range(4):
            nc.sync.dma_start(wgh[r][:], io["wgates_hi"][128 * r:128 * (r + 1), :])
            nc.sync.dma_start(wgl[r][:], io["wgates_lo"][128 * r:128 * (r + 1), :])
        nc.sync.dma_start(bias_fcv[:], io["bias_fcv"][:])
        nc.sync.dma_start(goffs[:], io["goffs"][:])
        nc.sync.dma_start(ident[:], io["ident"][:])
        nc.sync.dma_start(hT[:], io["h0t"][:])
        nc.sync.dma_start(c_t[:], io["c0"][:])
        nc.vector.tensor_copy(hTh[:], hT[:])
        nc.vector.tensor_tensor(hTl[:], hT[:], hTh[:], op=Alu.subtract)

        emb_g = io["emb_gates"]        # [V, 2048] fp32 (gate-permuted, +bias)
        out_logits = io["logits"]      # [T, 128, 1024]

        # xg: the x-contribution to the gates in pg layout [32q+b, 512]
        xg0 = sb.tile([128, 512], FP32, name="xg0")
        nc.sync.dma_start(xg0[:], io["xg0"][:])

        # remote-exchange receive buffer (fixed address, parity double-buffered)
        rx = sb.tile([128, 32], FP32, name="rx")
        # start barrier: CC AllGather; its output gates the first payload write
        barrier_sb = sb.tile([128, 2], FP32, name="barrier_sb")
        nc.vector.memset(barrier_sb[:], 0.0)
        zer8 = sb.tile([1, 8], FP32, name="zer8")
        nc.vector.memset(zer8[:], 0.0)
        bin_t = dr.tile([1, 8], FP32, name="bin_t")
        bout_t = dr.tile([8, 8], FP32, name="bout_t", addr_space="Shared")
        nc.sync.dma_start(bin_t[:], zer8[:])
        nc.gpsimd.collective_compute(
            "AllGather", Alu.bypass, replica_groups=[list(range(N_CORES))],
            ins=[bin_t[:]], outs=[bout_t[:]])
        nc.sync.dma_start(barrier_sb[0:2, 0:2], bout_t[0:2, 0:2])

        def emit_h_rounds(pg):
            # gates h-part: pg[32g+b, slot*128+hw] += h @ Whh.T (permuted cols)
            for r in range(4):
                cs = slice(32 * r, 32 * (r + 1))
                for g in range(4):
                    gs = slice(512 * g, 512 * (g + 1))
                    out = pg[32 * g:32 * (g + 1), :]
                    passes = ((hTh[:, cs], wgh[r][:, gs]),
                              (hTl[:, cs], wgh[r][:, gs]),
                              (hTh[:, cs], wgl[r][:, gs]))
                    for pi, (lhsT, rhs) in enumerate(passes):
                        nc.tensor.matmul(
                            out, lhsT=lhsT, rhs=rhs,
                            start=(r == 0 and pi == 0),
                            stop=(r == 3 and pi == 2),
                            tile_position=(0, 32 * g),
                            skip_group_check=True,
                        )

        xg_cur = xg0
        for t in range(n_steps):
            # ================= gates (h matmul + gathered x part) =================
            pg = ps_g.tile([128, 512], FP32, name="pg", tag="pg")
            emit_h_rounds(pg)
            gsum = sb2.tile([128, 512], FP32, name="gsum", tag="gsum")
            nc.vector.tensor_tensor(gsum[:], pg[:], xg_cur[:], op=Alu.add)

            # ================= activations / state =================
            # gate slots after host permutation: [i | f | o | g(tanh)]
            acts = sb2.tile([128, 512], FP32, name="acts", tag="acts")
            nc.scalar.activation(acts[:, 0:384], gsum[:, 0:384], Sigmoid)
            nc.scalar.activation(acts[:, 384:512], gsum[:, 384:512], Tanh)
            t1 = sb2.tile([128, 128], FP32, name="t1", tag="t1")
            nc.vector.tensor_tensor(t1[:], acts[:, 0:128], acts[:, 384:512], op=Alu.mult)
            nc.vector.tensor_tensor(c_t[:], acts[:, 128:256], c_t[:], op=Alu.mult)
            nc.vector.tensor_tensor(c_t[:], c_t[:], t1[:], op=Alu.add)
            tanh_c = sb2.tile([128, 128], FP32, name="tanh_c", tag="tanh_c")
            nc.scalar.activation(tanh_c[:], c_t[:], Tanh)
            h_new = sb2.tile([128, 128], FP32, name="h_new", tag="h_new")
            nc.vector.tensor_tensor(h_new[:], acts[:, 256:384], tanh_c[:], op=Alu.mult)

            # hT = transpose(h_new), then hi/lo split for bf16x3
            p_ht = ps_s.tile([128, 128], FP32, name="p_ht", tag="small")
            nc.tensor.transpose(p_ht[:], h_new[:], ident[:])
            nc.scalar.copy(hT[:], p_ht[:])
            nc.vector.tensor_copy(hTh[:], hT[:])
            nc.vector.tensor_tensor(hTl[:], hT[:], hTh[:], op=Alu.subtract)

            # ================= vocab matmul (+pipelined local argmax) ============
            # psum layout: partition 32g+b (g = vocab quarter of shard), free 1024
            pv = ps_v.tile([128, 1024], FP32, name="pv", tag="pv")
            staged = sb2.tile([128, 1024], FP32, name="staged", tag="staged")
            vh = sb2.tile([128, 2], FP32, name="vh", tag="vh")
            ih = sb2.tile([128, 2], U32, name="ih", tag="ih")
            v8 = [sb2.tile([128, 8], FP32, name=f"v8_{nt}", tag=f"v8_{nt}")
                  for nt in range(2)]
            i8 = [sb2.tile([128, 8], U32, name=f"i8_{nt}", tag=f"i8_{nt}")
                  for nt in range(2)]
            for nt in range(2):
                for q in range(4):
                    cs = slice(32 * q, 32 * (q + 1))
                    for g in range(4):
                        ws = slice(1024 * g + 512 * nt, 1024 * g + 512 * (nt + 1))
                        out = pv[32 * g:32 * (g + 1), 512 * nt:512 * (nt + 1)]
                        passes = ((hTh[:, cs], wfh[q][:, ws]),
                                  (hTl[:, cs], wfh[q][:, ws]),
                                  (hTh[:, cs], wfl[q][:, ws]))
                        for pi, (lhsT, rhs) in enumerate(passes):
                            nc.tensor.matmul(
                                out, lhsT=lhsT, rhs=rhs,
                                start=(q == 0 and pi == 0),
                                stop=(q == 3 and pi == 2),
                                tile_position=(0, 32 * g),
                                skip_group_check=True,
                            )
                # bias-add + per-half max/argmax while the other half matmuls
                hs = slice(512 * nt, 512 * (nt + 1))
                nc.vector.tensor_tensor(staged[:, hs], pv[:, hs],
                                        bias_fcv[:, hs], op=Alu.add)
                nc.vector.max(v8[nt][:], staged[:, hs])
                nc.vector.max_index(i8[nt][:], v8[nt][:], staged[:, hs])
                nc.vector.tensor_copy(vh[:, nt:nt + 1], v8[nt][:, 0:1])
                nc.vector.tensor_copy(ih[:, nt:nt + 1], i8[nt][:, 0:1])
            nc.scalar.dma_start(out_logits[t], staged[:])

            # combine the two halves; idx -> global vocab idx via goffs
            # goffs[:, nt] = VS*j + g*1024 + 512*nt  (per partition group g)
            ihf = sb2.tile([128, 2], FP32, name="ihf", tag="ihf")
            nc.vector.tensor_copy(ihf[:], ih[:])
            gidx = sb2.tile([128, 2], FP32, name="gidx", tag="gidx")
            nc.vector.tensor_tensor(gidx[:], ihf[:], goffs[:], op=Alu.add)
            vmax = sb2.tile([128, 1], FP32, name="vmax", tag="vmax")
            nc.vector.tensor_reduce(vmax[:], vh[:], axis=mybir.AxisListType.X, op=Alu.max)
            eqh = sb2.tile([128, 2], FP32, name="eqh", tag="eqh")
            nc.vector.tensor_scalar(eqh[:], vh[:], vmax[:, 0:1], None, op0=Alu.is_equal)
            lmih = sb2.tile([128, 2], FP32, name="lmih", tag="lmih")
            nc.vector.tensor_scalar(lmih[:], gidx[:], -1.0, BIG, op0=Alu.mult, op1=Alu.add)
            mselh = sb2.tile([128, 2], FP32, name="mselh", tag="mselh")
            nc.vector.tensor_tensor(mselh[:], eqh[:], lmih[:], op=Alu.mult)
            pay = sb2.tile([128, 2], FP32, name="pay", tag="pay")
            # pay[:, 1] = BIG - sel -> global idx of the local winner
            nc.vector.tensor_reduce(pay[:, 1:2], mselh[:], axis=mybir.AxisListType.X,
                                    op=Alu.max)
            nc.vector.tensor_scalar(pay[:, 1:2], pay[:, 1:2], -1.0, BIG,
                                    op0=Alu.mult, op1=Alu.add)
            nc.vector.tensor_copy(pay[:, 0:1], vmax[:])

            # ================= remote-DMA exchange =================
            # Each core broadcasts its per-(g,b) candidates [128, 2] to all 7
            # peers' rx columns; peer identity is XOR-relative (column = xor
            # delta, fold is permutation-invariant). One trigger fires all.
            par = t % 2
            roff = 16 * par
            if t == 0:
                # pin the first sends behind the start barrier
                nc.vector.tensor_tensor(pay[:], pay[:], barrier_sb[:], op=Alu.add)
            nc.vector.tensor_copy(rx[:, roff:roff + 2], pay[:])
            for dd in range(1, 8):
                rdests = [None] * 8
                rdests[dd] = (0, dd)
                nc.gpsimd.remote_dma_broadcast(
                    rx[:, roff + 2 * dd:roff + 2 * dd + 2], pay[:],
                    remote_sem, local_sem, rdests=rdests)
            nc.gpsimd.trigger_dma(count=None).then_inc(trig_sem, 1)

            # ================= global fold =================
            # ranks on the free dim first (the deferred remote_sem wait rides
            # on this reduce), then the 4 partition groups via PE transposes.
            rvals = rx[:, roff:roff + 16:2]
            ridxs = rx[:, roff + 1:roff + 16:2]
            gvr = sb2.tile([128, 1], FP32, name="gvr", tag="gvr")
            red0 = nc.vector.tensor_reduce(gvr[:], rvals, axis=mybir.AxisListType.X,
                                           op=Alu.max)
            deferred.append((red0, remote_sem, 14 * (t + 1)))
            eqr = sb2.tile([128, 8], FP32, name="eqr", tag="eqr")
            er = nc.vector.tensor_scalar(eqr[:], rvals, gvr[:, 0:1], None,
                                         op0=Alu.is_equal)
            deferred.append((er, remote_sem, 14 * (t + 1)))
            lmir = sb2.tile([128, 8], FP32, name="lmir", tag="lmir")
            lr = nc.vector.tensor_scalar(lmir[:], ridxs, -1.0, BIG,
                                         op0=Alu.mult, op1=Alu.add)
            deferred.append((lr, remote_sem, 14 * (t + 1)))
            mselr = sb2.tile([128, 8], FP32, name="mselr", tag="mselr")
            nc.vector.tensor_tensor(mselr[:], eqr[:], lmir[:], op=Alu.mult)
            rbest = sb2.tile([128, 2], FP32, name="rbest", tag="rbest")
            nc.vector.tensor_reduce(rbest[:, 1:2], mselr[:],
                                    axis=mybir.AxisListType.X, op=Alu.max)
            nc.vector.tensor_scalar(rbest[:, 1:2], rbest[:, 1:2], -1.0, BIG,
                                    op0=Alu.mult, op1=Alu.add)
            nc.vector.tensor_copy(rbest[:, 0:1], gvr[:])

            # fold over the 4 partition groups (g): first-occurrence argmax
            p_pa = ps_s.tile([2, 128], FP32, name="p_pa", tag="small")
            nc.tensor.transpose(p_pa[:], rbest[:, 0:1].to_broadcast([128, 2]), ident[:])
            vboth = sb2.tile([2, 128], FP32, name="vboth", tag="vboth")
            nc.scalar.copy(vboth[:], p_pa[:])
            p_pb = ps_s.tile([2, 128], FP32, name="p_pb", tag="small")
            nc.tensor.transpose(p_pb[:], rbest[:], ident[:])
            payT = sb2.tile([2, 128], FP32, name="payT", tag="payT")
            nc.vector.tensor_copy(payT[:], p_pb[:])  # DVE, parallel to ACT's vboth copy

            vb3 = vboth[:].rearrange("p (g b) -> p b g", g=4)
            m4 = sb2.tile([2, 32], FP32, name="m4", tag="m4")
            nc.vector.tensor_reduce(m4[:], vb3, axis=mybir.AxisListType.X, op=Alu.max)
            eq = sb2.tile([2, 128], FP32, name="eq", tag="eq")
            nc.vector.tensor_tensor(
                eq[:].rearrange("p (g b) -> p b g", g=4), vb3,
                m4[:].to_broadcast([2, 32, 4]), op=Alu.is_equal)
            lmi = sb2.tile([2, 128], FP32, name="lmi", tag="lmi")
            nc.vector.tensor_scalar(lmi[:], payT[:], -1.0, BIG, op0=Alu.mult, op1=Alu.add)
            msel = sb2.tile([2, 128], FP32, name="msel", tag="msel")
            nc.vector.tensor_tensor(msel[:], eq[:], lmi[:], op=Alu.mult)
            res = sb2.tile([2, 32], FP32, name="res", tag="res")
            nc.vector.tensor_reduce(
                res[:], msel[:].rearrange("p (g b) -> p b g", g=4),
                axis=mybir.AxisListType.X, op=Alu.max)
            # winner idx per b: BIG - res (row 1); transpose [2,32] -> [32,2]
            idxg = sb2.tile([2, 32], FP32, name="idxg", tag="idxg")
            nc.vector.tensor_scalar(idxg[:], res[:], -1.0, BIG,
                                    op0=Alu.mult, op1=Alu.add)
            p_ti = ps_s.tile([32, 2], FP32, name="p_ti", tag="small")
            nc.tensor.transpose(p_ti[:], idxg[:], ident[0:2, 0:2])
            idx32 = sb2.tile([32, 1], I32, name="idx32", tag="idx32")
            nc.vector.tensor_copy(idx32[:], p_ti[:, 1:2])

            # ================= next-step x gates gather =================
            if t < n_steps - 1:
                xg32 = sb2.tile([32, 2048], FP32, name="xg32", tag="xg32")
                gi = nc.gpsimd.indirect_dma_start(
                    out=xg32[:], out_offset=None, in_=emb_g[:],
                    in_offset=bass.IndirectOffsetOnAxis(ap=idx32[:, 0:1], axis=0),
                )
                # visible pin: the gather (and anything later on gpsimd) must
                # not be scheduled ahead of this step's trigger
                gi.wait_op(trig_sem, t + 1, "sem-ge", check=False)
                xg = sb2.tile([128, 512], FP32, name="xg", tag="xg")
                for q in range(4):
                    nc.sync.dma_start(xg[32 * q:32 * (q + 1), :],
                                      xg32[:, 512 * q:512 * (q + 1)])
                xg_cur = xg

        endg_t = sb.tile([32, 1], FP32, name="endg_t")
        endgate = nc.gpsimd.tensor_copy(endg_t[:], idx32[:])
        deferred.append((endgate, remote_sem, 14 * n_steps))

        for p in reversed(pools):
            p.release()
    return deferred


def host_prep(inputs):
    """Build per-core in_maps from the full problem inputs."""
    z = np.asarray(inputs["z"], np.float32)
    embedding = np.asarray(inputs["embedding"], np.float32)
    Wh = np.asarray(inputs["Wh"], np.float32)
    bh = np.asarray(inputs["bh"], np.float32)
    Wc = np.asarray(inputs["Wc"], np.float32)
    bc = np.asarray(inputs["bc"], np.float32)
    Wih = np.asarray(inputs["Wih"], np.float32)
    Whh = np.asarray(inputs["Whh"], np.float32)
    bih = np.asarray(inputs["bih"], np.float32)
    bhh = np.asarray(inputs["bhh"], np.float32)
    Wfc = np.asarray(inputs["Wfc"], np.float32)
    bfc = np.asarray(inputs["bfc"], np.float32)

    h0 = (z @ Wh.T + bh).astype(np.float32)   # [B, H]
    c0 = (z @ Wc.T + bc).astype(np.float32)
    b_gates = (bih + bhh).astype(np.float32)  # [4H]

    # gate column permutation: c' = q*512 + slot*128 + hw with slot order
    # [i, f, o, g] so the sigmoid gates are one contiguous 384-wide range.
    cp = np.arange(2048)
    qq, rem = cp // 512, cp % 512
    slot, hw = rem // 128, rem % 128
    gate = np.array([0, 1, 3, 2])[slot]        # slot -> original gate (i,f,o,g)
    perm = gate * 512 + qq * 128 + hw          # original col index for permuted col c'

    Wperm_h = Whh[perm]                        # [2048, 512]
    wgates = np.ascontiguousarray(Wperm_h.T)   # [512, 2048]

    def split_bf16(w):
        import ml_dtypes
        hi = w.astype(ml_dtypes.bfloat16)
        lo = (w - hi.astype(np.float32)).astype(ml_dtypes.bfloat16)
        return np.ascontiguousarray(hi), np.ascontiguousarray(lo)

    wgates_hi, wgates_lo = split_bf16(wgates)

    # x-path precompute: emb_gates[v, c'] = (embedding @ Wih.T + b)[v, perm[c']]
    eg = embedding @ Wih.T + b_gates           # [V, 2048] fp32
    emb_gates = np.ascontiguousarray(eg[:, perm], np.float32)
    xg0_rows = (h0 @ Wih.T + b_gates)[:, perm]  # [B, 2048]
    # pg layout: xg0[32q+b, f] = xg0_rows[b, 512q + f]
    xg0 = np.zeros((128, 512), np.float32)
    for q in range(4):
        xg0[32 * q:32 * (q + 1), :] = xg0_rows[:, 512 * q:512 * (q + 1)]

    # state layout tiles
    h0t = np.zeros((128, 128), np.float32)     # h0t[p, q*32+b] = h0[b, 128q+p]
    c0t = np.zeros((128, 128), np.float32)     # c0t[32q+b, hw] = c0[b, 128q+hw]
    for q in range(4):
        h0t[:, 32 * q:32 * (q + 1)] = h0[:, 128 * q:128 * (q + 1)].T
        c0t[32 * q:32 * (q + 1), :] = c0[:, 128 * q:128 * (q + 1)]

    ident = np.eye(128, dtype=np.float32)

    in_maps = []
    for j in range(N_CORES):
        shard = Wfc[VS * j:VS * (j + 1)]                    # [4000, 512]
        shard_p = np.zeros((VSP, H), np.float32)
        shard_p[:VS] = shard
        wfc_in = np.ascontiguousarray(shard_p.T)            # [512, 4096]
        wfc_hi, wfc_lo = split_bf16(wfc_in)
        bfc_p = np.full(VSP, -1e30, np.float32)
        bfc_p[:VS] = bfc[VS * j:VS * (j + 1)]
        # staged-layout bias: bias_fcv[32g+b, v] = bfc_p[1024g + v]
        bias_fcv = np.repeat(bfc_p.reshape(4, 1, 1024), 32, axis=1).reshape(128, 1024)
        # per-half global idx offsets: goffs[32g+b, nt] = VS*j + 1024*g + 512*nt
        goffs = (VS * j + (np.arange(128) // 32) * 1024).astype(np.float32)[:, None]
        goffs = np.concatenate([goffs, goffs + 512.0], axis=1)  # [128, 2]
        in_maps.append({
            "wfc_hi": wfc_hi,
            "wfc_lo": wfc_lo,
            "wgates_hi": wgates_hi,
            "wgates_lo": wgates_lo,
            "bias_fcv": np.ascontiguousarray(bias_fcv),
            "goffs": np.ascontiguousarray(goffs),
            "ident": ident,
            "h0t": h0t,
            "c0": c0t,
            "xg0": xg0,
            "emb_gates": emb_gates,
        })
    return in_maps


def declare_io(nc, n_steps):
    io = {}
    io["wfc_hi"] = nc.dram_tensor("wfc_hi", [512, VSP], BF16, kind="ExternalInput").ap()
    io["wfc_lo"] = nc.dram_tensor("wfc_lo", [512, VSP], BF16, kind="ExternalInput").ap()
    io["wgates_hi"] = nc.dram_tensor("wgates_hi", [512, 2048], BF16, kind="ExternalInput").ap()
    io["wgates_lo"] = nc.dram_tensor("wgates_lo", [512, 2048], BF16, kind="ExternalInput").ap()
    io["bias_fcv"] = nc.dram_tensor("bias_fcv", [128, 1024], FP32, kind="ExternalInput").ap()
    io["goffs"] = nc.dram_tensor("goffs", [128, 2], FP32, kind="ExternalInput").ap()
    io["ident"] = nc.dram_tensor("ident", [128, 128], FP32, kind="ExternalInput").ap()
    io["h0t"] = nc.dram_tensor("h0t", [128, 128], FP32, kind="ExternalInput").ap()
    io["c0"] = nc.dram_tensor("c0", [128, 128], FP32, kind="ExternalInput").ap()
    io["xg0"] = nc.dram_tensor("xg0", [128, 512], FP32, kind="ExternalInput").ap()
    io["emb_gates"] = nc.dram_tensor("emb_gates", [V, 2048], FP32, kind="ExternalInput").ap()
    io["logits"] = nc.dram_tensor("logits", [n_steps, 128, 1024], FP32,
                                  kind="ExternalOutput").ap()
    return io


_BUILT = {}


def build(n_steps=T):
    if n_steps in _BUILT:
        return _BUILT[n_steps]
    nc = bacc.Bacc("TRN2", target_bir_lowering=False, debug=False,
                   num_devices=N_CORES)
    io = declare_io(nc, n_steps)
    deferred = build_decoder(nc, io, n_steps)
    for inst, sem, val in deferred:
        inst.wait_op(sem, val, "sem-ge", check=False)
    nc.compile()
    _BUILT[n_steps] = nc
    return nc


def assemble(results, n_steps=T):
    """results: list of per-core out dicts -> full [B, T, V] fp32."""
    full = np.empty((B, n_steps, V), np.float32)
    for j in range(N_CORES):
        arr = results[j]["logits"].reshape(n_steps, 4, 32, 1024)
        arr = arr.transpose(2, 0, 1, 3).reshape(B, n_steps, VSP)[:, :, :VS]
        full[:, :, VS * j:VS * (j + 1)] = arr
    return full


def kernel(**inputs):
    n_steps = int(inputs.get("context_length", T))
    assert n_steps == T, f"kernel hardcodes T={T}, got {n_steps}"
    nc = build(T)
    in_maps = host_prep(inputs)
    res = run_bass_kernel_spmd(nc, in_maps, core_ids=list(range(N_CORES)))
    return assemble(res.results, T)


if __name__ == "__main__":
    import reference
    inputs = reference.setup_inputs()
    out = kernel(**{k: np.asarray(v) if hasattr(v, "shape") else v
                    for k, v in inputs.items()})
    print("output shape:", out.shape)
